# revision 1
# baseline (speedup 1.0000x reference)
"""Trainium2 Bass kernel for a dense transformer encoder layer.

Shapes (hardcoded): B=2, L=2048, D=1024, F=4096, H=16 heads, dk=64.
Sharding over 8 NeuronCores: core c handles batch b=c//4 and query-row
quarter r=c%4 (512 rows). K/V projections for the full batch are computed
per core (replicated within the 4-core batch group) so no collectives are
needed. All matmuls run in bf16 with fp32 PSUM accumulation.
"""
import os
import sys
import types

sys.path.insert(0, "/opt/trn_rl_repo")

import numpy as np
import ml_dtypes

import concourse.bass as bass
import concourse.tile as tile
import concourse.mybir as mybir
from contextlib import ExitStack

f32 = mybir.dt.float32
bf16 = mybir.dt.bfloat16
AF = mybir.ActivationFunctionType
ALU = mybir.AluOpType

B, L, D, F, H, DK = 2, 2048, 1024, 4096, 16, 64
RQ = 512          # query rows per core
NCORES = 8
EPS = 1e-6

_PATCHED = False


def _install_patches():
    """Register the NTFF profile hook (if available) and wrap the BIR
    compile step to split multi-wait instructions (this walrus build
    accepts at most one sync-wait per instruction)."""
    global _PATCHED
    if _PATCHED:
        return
    _PATCHED = True

    # NTFF profile hook (for trace=True); degrade silently if absent.
    if "antenv.axon_hooks" not in sys.modules:
        try:
            from trn_agent_boot.trn_boot import _ntff_profile_via_ctypes
            hook = _ntff_profile_via_ctypes("/opt/axon/libaxon_pjrt.so")
        except Exception:
            hook = None
        mod = types.ModuleType("antenv.axon_hooks")
        mod.get_axon_ntff_profile_hook = lambda: hook
        mod.set_axon_ntff_profile_hook = lambda h: None
        sys.modules["antenv.axon_hooks"] = mod

    import json

    def _split_multiwaits(bir_bytes):
        d = json.loads(bir_bytes)
        ctr = 0
        for fn in d.get("functions", []):
            for blk in fn.get("blocks", []):
                out = []
                for inst in blk.get("instructions", []):
                    si = inst.get("sync_info")
                    ow = (si or {}).get("on_wait") or []
                    if len(ow) > 1 and inst.get("engine", "Unassigned") != "Unassigned":
                        for w in ow[:-1]:
                            out.append({
                                "debug": inst.get("debug", 0),
                                "engine": inst["engine"],
                                "ins": [], "outs": [],
                                "name": f"I-antsw{ctr}",
                                "opcode": "NoOp",
                                "sync_info": {"on_update": [], "on_wait": [w]},
                            })
                            ctr += 1
                        si["on_wait"] = [ow[-1]]
                    out.append(inst)
                blk["instructions"] = out
        return json.dumps(d).encode()

    import concourse.bass_utils as bu
    import concourse.bass2jax as b2j

    orig = bu.compile_bir_kernel

    def patched(bir_json, tmpdir, neff_name="file.neff"):
        return orig(_split_multiwaits(bir_json), tmpdir, neff_name=neff_name)

    bu.compile_bir_kernel = patched
    b2j.compile_bir_kernel = patched

    if os.environ.get("ANT_LDW_OPT"):
        orig_run = bu.run_command

        def run_ldw(argv, **kw):
            argv = [a.replace("--enable-ldw-opt=false", "--enable-ldw-opt=true")
                    for a in argv]
            return orig_run(argv, **kw)

        bu.run_command = run_ldw


def _build_program(flags):
    """Build the SPMD Bass program (same NEFF for all 8 cores)."""
    nc = bass.Bass("TRN2", target_bir_lowering=False, debug=False,
                   num_devices=NCORES)

    def din(name, shape, dt):
        return nc.dram_tensor(name, shape, dt, kind="ExternalInput").ap()

    xT = din("xT", [D, L], bf16)            # batch x, transposed
    xTq = din("xTq", [D, RQ], bf16)         # this core's columns of x[b].T
    xr = din("xr", [RQ, D], f32)            # this core's rows (residual)
    cosr = din("cosr", [128, L], bf16)      # cos table, [p%32] replicated
    sinr = din("sinr", [128, L], bf16)      # sign-baked sin table
    qcos = din("qcos", [128, RQ], bf16)     # cos table slice for these rows
    qsin = din("qsin", [128, RQ], bf16)     # sign-baked sin slice
    wq = din("wq", [D, D], bf16)
    wk = din("wk", [D, D], bf16)
    wv = din("wv", [D, D], bf16)
    wo = din("wo", [D, D], bf16)
    w1 = din("w1", [D, F], bf16)
    w2 = din("w2", [F, D], bf16)
    b1t = din("b1t", [128, F // 128], f32)  # b1 reshaped per-partition
    ident = din("ident", [128, 128], f32)
    onehot = din("onehot", [H, H * 64], bf16)
    bo = din("bo", [1, D], f32)
    b2r = din("b2r", [1, D], f32)
    g1 = din("g1", [1, D], f32)
    be1 = din("be1", [1, D], f32)
    g2 = din("g2", [1, D], f32)
    be2 = din("be2", [1, D], f32)
    y = nc.dram_tensor("y", [RQ, D], f32, kind="ExternalOutput").ap()

    VSTR = 65           # per-head stride in v_ext (64 v cols + ones)
    KT = D // 128       # 8 contraction tiles over D
    NL = L // 512       # 4 free chunks over L
    LT = L // 128       # 16 l-tiles
    FT = F // 128       # 32 f-tiles

    def bcast_ap(ap2d, width):
        return bass.AP(tensor=ap2d.tensor, offset=ap2d.offset,
                       ap=[[0, 128], [1, width]])

    with tile.TileContext(nc) as tc:
      with ExitStack() as top:
        # Pools are a stack allocator: open in reverse order of release.
        consts = top.enter_context(tc.tile_pool(name="consts", bufs=1))
        poolD = top.enter_context(tc.tile_pool(name="pd", bufs=1))   # ff1rT
        poolC1 = top.enter_context(tc.tile_pool(name="pc1", bufs=1))  # hT
        poolB = top.enter_context(tc.tile_pool(name="pb", bufs=1))    # ctxT
        stackA = ExitStack()                                          # kTr/qTr/v_ext
        poolA = stackA.enter_context(tc.tile_pool(name="pa", bufs=1))

        # --- long-lived constants ---
        ident_sb = consts.tile([128, 128], f32, tag="ident", name="ident")
        nc.sync.dma_start(ident_sb[:], ident[:])
        b1_sb = consts.tile([128, F // 128], f32, tag="b1", name="b1")
        nc.sync.dma_start(b1_sb[:], b1t[:])
        onehot_sb = consts.tile([H, H * 64], bf16, tag="onehot", name="onehot")
        nc.sync.dma_start(onehot_sb[:], onehot[:])
        eps_sb = consts.tile([128, 1], f32, tag="eps", name="eps")
        nc.vector.memset(eps_sb[:], EPS)

        def rep_const(ap2d, use, tag):
            if not use:
                return None
            t = consts.tile([128, D], f32, tag=tag, name=tag)
            nc.gpsimd.dma_start(out=t[:], in_=bcast_ap(ap2d, D))
            return t

        bo_rep = rep_const(bo, flags["use_bo"], "bo")
        b2_rep = rep_const(b2r, flags["use_b2"], "b2")
        g1_rep = rep_const(g1, flags["use_g1"], "g1")
        be1_rep = rep_const(be1, flags["use_be1"], "be1")
        g2_rep = rep_const(g2, flags["use_g2"], "g2")
        be2_rep = rep_const(be2, flags["use_be2"], "be2")

        # persistent activations
        ff1rT = [poolD.tile([128, RQ], bf16, tag=f"ff1{t}", name=f"ff1{t}")
                 for t in range(FT)]
        hT = [poolC1.tile([128, RQ], bf16, tag=f"hT{k}", name=f"hT{k}")
              for k in range(KT)]
        ctxT = [poolB.tile([128, RQ], bf16, tag=f"ctxT{m}", name=f"ctxT{m}")
                for m in range(KT)]
        kTr = [poolA.tile([128, L], bf16, tag=f"kTr{m}", name=f"kTr{m}")
               for m in range(KT)]
        qTr = [poolA.tile([128, RQ], bf16, tag=f"qTr{m}", name=f"qTr{m}")
               for m in range(KT)]
        v_ext = [poolA.tile([128, H * VSTR], bf16, tag=f"vx{t}", name=f"vx{t}")
                 for t in range(LT)]
        # DRAM spill for h (residual into LN2)
        with tc.tile_pool(name="dramp", bufs=1, space="DRAM") as dramp:
            h_dram = dramp.tile([RQ, D], f32, tag="h_dram", name="h_dram")


            # ================= Phase 1: projections =================
            with tc.tile_pool(name="ph1c", bufs=1) as ph1c, \
                 tc.tile_pool(name="ph1w", bufs=3) as ph1w:
                qcos_sb = ph1c.tile([128, RQ], bf16, tag="qcos", name="qcos")
                nc.sync.dma_start(qcos_sb[:], qcos[:])
                qsin_sb = ph1c.tile([128, RQ], bf16, tag="qsin", name="qsin")
                nc.sync.dma_start(qsin_sb[:], qsin[:])
                xTq_sb = [ph1c.tile([128, RQ], bf16, tag=f"xTq{k}", name=f"xTq{k}")
                          for k in range(KT)]
                for k in range(KT):
                    nc.sync.dma_start(xTq_sb[k][:], xTq[k * 128:(k + 1) * 128, :])

                def rope_chunk(ps, cos_sl, sinsw_sl, dst):
                    """dst = ps*cos + swap32(ps*sinsw).

                    sinsw is the sign-baked sin table PRE-INDEXED by the
                    partner row, so the partition swap happens on the
                    product via a small SBUF-to-SBUF DMA."""
                    n = dst.shape[-1]
                    tct = ph1w.tile([128, 512], bf16, tag="rtc", name="rtc")
                    nc.vector.tensor_mul(tct[:, :n], ps, cos_sl)
                    tsn = ph1w.tile([128, 512], bf16, tag="rtm", name="rtm")
                    nc.vector.tensor_mul(tsn[:, :n], ps, sinsw_sl)
                    tsw = ph1w.tile([128, 512], bf16, tag="tsw", name="tsw")
                    for g in range(2):
                        o = g * 64
                        nc.sync.dma_start(tsw[o:o + 32, :n], tsn[o + 32:o + 64, :n])
                        nc.sync.dma_start(tsw[o + 32:o + 64, :n], tsn[o:o + 32, :n])
                    nc.vector.tensor_add(dst, tct[:, :n], tsw[:, :n])

                # kv weights + tables load from t=0 while q-projection runs
                with tc.tile_pool(name="wkv", bufs=1) as wkv, \
                     tc.tile_pool(name="xcp", bufs=1) as xcp:
                    cos_sb = wkv.tile([128, L], bf16, tag="cos", name="cos")
                    nc.sync.dma_start(cos_sb[:], cosr[:])
                    sin_sb = wkv.tile([128, L], bf16, tag="sin", name="sin")
                    nc.sync.dma_start(sin_sb[:], sinr[:])
                    wk_sb = [wkv.tile([128, D], bf16, tag=f"wk{k}", name=f"wk{k}")
                             for k in range(KT)]
                    wv_sb = [wkv.tile([128, D], bf16, tag=f"wv{k}", name=f"wv{k}")
                             for k in range(KT)]
                    for k in range(KT):
                        nc.scalar.dma_start(wk_sb[k][:], wk[k * 128:(k + 1) * 128, :])

                    # kT + v for the full batch, xT streamed per 512-col chunk
                    with tc.tile_pool(name="ph1ps", bufs=3, space="PSUM") as ph1ps, \
                         tc.tile_pool(name="wqp", bufs=1) as wqp:
                        # q projection first: m-outer, shares the kT psum pool
                        wq_sb = [wqp.tile([128, D], bf16, tag=f"wq{k}",
                                          name=f"wq{k}") for k in range(KT)]
                        for k in range(KT):
                            nc.sync.dma_start(wq_sb[k][:],
                                              wq[k * 128:(k + 1) * 128, :])
                        for m in range(KT):
                            msl = slice(m * 128, m * 128 + 128)
                            ps = ph1ps.tile([128, 512], f32, tag="pj", name="pj")
                            for k in range(KT):
                                nc.tensor.matmul(ps[:], wq_sb[k][:, msl],
                                                 xTq_sb[k][:],
                                                 start=(k == 0), stop=(k == KT - 1))
                            rope_chunk(ps[:], qcos_sb[:], qsin_sb[:], qTr[m][:])

                        for n in range(NL):
                            nsl = slice(n * 512, n * 512 + 512)
                            xc = [xcp.tile([128, 512], bf16, tag=f"xc{k}",
                                           name=f"xc{k}") for k in range(KT)]
                            for k in range(KT):
                                nc.scalar.dma_start(
                                    xc[k][:], xT[k * 128:(k + 1) * 128, nsl])
                            for m in range(KT):
                                msl = slice(m * 128, m * 128 + 128)
                                ps = ph1ps.tile([128, 512], f32, tag="pj", name="pj")
                                for k in range(KT):
                                    nc.tensor.matmul(
                                        ps[:], wk_sb[k][:, msl], xc[k][:],
                                        start=(k == 0), stop=(k == KT - 1))
                                rope_chunk(ps[:], cos_sb[:, nsl], sin_sb[:, nsl],
                                           kTr[m][:, nsl])
                            if n == 0:
                                for k in range(KT):
                                    nc.scalar.dma_start(
                                        wv_sb[k][:],
                                        wv[k * 128:(k + 1) * 128, :])
                            for j in range(4):
                                t = n * 4 + j
                                tsl = slice(j * 128, j * 128 + 128)
                                vx_view = v_ext[t][:].rearrange(
                                    "p (h e) -> p h e", h=H)
                                for n2 in range(2):
                                    ps = ph1ps.tile([128, 512], f32, tag="pv",
                                                    name="pv")
                                    for k in range(KT):
                                        nc.tensor.matmul(
                                            ps[:], xc[k][:, tsl],
                                            wv_sb[k][:, n2 * 512:n2 * 512 + 512],
                                            start=(k == 0), stop=(k == KT - 1))
                                    ps_view = ps[:].rearrange(
                                        "p (h e) -> p h e", h=8)
                                    nc.vector.tensor_copy(
                                        vx_view[:, n2 * 8:n2 * 8 + 8, 0:DK],
                                        ps_view[:])
                                nc.vector.memset(vx_view[:, :, DK:DK + 1], 1.0)

            # ================= Phase 2: attention =================
            with tc.tile_pool(name="ph2", bufs=1) as ph2, \
                 tc.tile_pool(name="ph2e", bufs=5) as ph2e, \
                 tc.tile_pool(name="ph2w", bufs=3) as ph2w:
                ctxraw = [ph2.tile([65, RQ], f32, tag=f"cr{h}", name=f"cr{h}")
                          for h in range(H)]
                rec2b = []
                with tc.tile_pool(name="scps", bufs=3, space="PSUM") as scps, \
                     tc.tile_pool(name="ctxps", bufs=1, space="PSUM") as ctxps:
                    for hp in range(KT):  # head pairs
                        hA, hB = 2 * hp, 2 * hp + 1
                        cpsA = ctxps.tile([65, RQ], f32, tag="cpsA", name="cpsA")
                        cpsB = ctxps.tile([65, RQ], f32, tag="cpsB", name="cpsB")
                        for kt in range(LT):
                            ksl = slice(kt * 128, kt * 128 + 128)
                            sc = scps.tile([128, 2 * RQ], f32, tag="sc", name="sc")
                            nc.tensor.matmul(sc[:, 0:RQ], kTr[hp][0:64, ksl],
                                             qTr[hp][0:64, :], start=True, stop=True)
                            nc.tensor.matmul(sc[:, RQ:2 * RQ], kTr[hp][64:128, ksl],
                                             qTr[hp][64:128, :], start=True, stop=True)
                            e = ph2e.tile([128, 2 * RQ], bf16, tag="e", name="e")
                            nc.scalar.activation(e[:], sc[:], AF.Exp, scale=0.125)
                            nc.tensor.matmul(
                                cpsA[:], v_ext[kt][:, hA * VSTR:hA * VSTR + 65],
                                e[:, 0:RQ], start=(kt == 0), stop=(kt == LT - 1))
                            nc.tensor.matmul(
                                cpsB[:], v_ext[kt][:, hB * VSTR:hB * VSTR + 65],
                                e[:, RQ:2 * RQ], start=(kt == 0), stop=(kt == LT - 1))
                        nc.vector.tensor_copy(ctxraw[hA][:], cpsA[0:65, :])
                        nc.vector.tensor_copy(ctxraw[hB][:], cpsB[0:65, :])
                        # denominator reciprocal per pair (no PSUM involved)
                        s2 = ph2.tile([2, RQ], f32, tag=f"s2_{hp}",
                                      name=f"s2_{hp}")
                        nc.sync.dma_start(s2[0:1, :], ctxraw[hA][64:65, :])
                        nc.sync.dma_start(s2[1:2, :], ctxraw[hB][64:65, :])
                        rec2 = ph2.tile([2, RQ], f32, tag=f"rc_{hp}",
                                        name=f"rc_{hp}")
                        nc.vector.reciprocal(rec2[:], s2[:])
                        rec2b.append(ph2.tile([2, RQ], bf16, tag=f"rb_{hp}",
                                              name=f"rb_{hp}"))
                        nc.vector.tensor_copy(rec2b[hp][:], rec2[:])

                # broadcast + rescale into ctxT
                with tc.tile_pool(name="rbps", bufs=2, space="PSUM") as rbps:
                    for h in range(H):
                        hp, half = h // 2, h % 2
                        rp = rbps.tile([64, RQ], f32, tag="rp", name="rp")
                        nc.tensor.matmul(
                            rp[:], onehot_sb[0:2, half * 64:half * 64 + 64],
                            rec2b[hp][:], start=True, stop=True)
                        dst = ctxT[hp][half * 64:half * 64 + 64, :]
                        nc.vector.tensor_mul(dst, ctxraw[h][0:64, :], rp[:])

            stackA.close()
            stackW1 = ExitStack()
            w1pp = stackW1.enter_context(tc.tile_pool(name="w1pp", bufs=1))
            w1pre = [w1pp.tile([128, F], bf16, tag=f"w1p{k}", name=f"w1p{k}")
                     for k in range(4)]
            for k in range(4):
                nc.sync.dma_start(w1pre[k][:], w1[k * 128:(k + 1) * 128, :])

            # ================= Phase 3: w_o + residual + LN1 =================
            def layer_norm(dst, src, g_rep, be_rep, wpool):
                sview = src.rearrange("p (s d) -> p s d", s=2)
                stats = wpool.tile([128, 2, 6], f32, tag="lnstats", name="lnstats")
                for s in range(2):
                    nc.vector.bn_stats(stats[:, s, :], sview[:, s, :])
                mv = wpool.tile([128, 2], f32, tag="lnmv", name="lnmv")
                nc.vector.bn_aggr(mv[:], stats[:])
                std = wpool.tile([128, 1], f32, tag="lnstd", name="lnstd")
                nc.scalar.activation(std[:], mv[:, 1:2], AF.Sqrt, bias=eps_sb[:])
                rstd = wpool.tile([128, 1], f32, tag="lnrstd", name="lnrstd")
                nc.vector.reciprocal(rstd[:], std[:])
                nc.vector.tensor_scalar(dst, src, mv[:, 0:1], rstd[:],
                                        op0=ALU.subtract, op1=ALU.mult)
                if g_rep is not None:
                    nc.vector.tensor_mul(dst, dst, g_rep[:])
                if be_rep is not None:
                    nc.vector.tensor_add(dst, dst, be_rep[:])

            with tc.tile_pool(name="ph3w", bufs=3) as ph3w, \
                 tc.tile_pool(name="ph3c", bufs=1) as ph3c, \
                 tc.tile_pool(name="aops", bufs=2, space="PSUM") as aops, \
                 tc.tile_pool(name="tpps", bufs=4, space="PSUM") as tpps:
                wo_sb = [ph3c.tile([128, D], bf16, tag=f"wo{k}", name=f"wo{k}")
                         for k in range(KT)]
                for k in range(KT):
                    nc.sync.dma_start(wo_sb[k][:], wo[k * 128:(k + 1) * 128, :])
                xr_sb = [ph3c.tile([128, D], f32, tag=f"xr{t}", name=f"xr{t}")
                         for t in range(4)]
                for t in range(4):
                    nc.sync.dma_start(xr_sb[t][:], xr[t * 128:(t + 1) * 128, :])
                for qt in range(4):
                    qsl = slice(qt * 128, qt * 128 + 128)
                    ps = aops.tile([128, D], f32, tag="ao", name="ao")
                    for half in range(2):
                        osl = slice(half * 512, half * 512 + 512)
                        for m in range(KT):
                            nc.tensor.matmul(ps[:, osl], ctxT[m][:, qsl],
                                             wo_sb[m][:, osl],
                                             start=(m == 0), stop=(m == KT - 1))
                    res = ph3w.tile([128, D], f32, tag="res", name="res")
                    nc.vector.tensor_add(res[:], ps[:], xr_sb[qt][:])
                    if bo_rep is not None:
                        nc.vector.tensor_add(res[:], res[:], bo_rep[:])
                    hq = ph3w.tile([128, D], f32, tag="hq", name="hq")
                    layer_norm(hq[:], res[:], g1_rep, be1_rep, ph3w)
                    nc.sync.dma_start(h_dram[qsl, :], hq[:])
                    for m in range(KT):
                        tp = tpps.tile([128, 128], f32, tag="tp", name="tp")
                        nc.tensor.transpose(tp[:], hq[:, m * 128:m * 128 + 128],
                                            ident_sb[:])
                        nc.vector.tensor_copy(hT[m][:, qsl], tp[:])

            # ================= Phase 4a: FFN up + ReLU =================
            with tc.tile_pool(name="ph4a", bufs=1) as ph4a, \
                 tc.tile_pool(name="f1ps", bufs=4, space="PSUM") as f1ps:
                w1_sb = w1pre + [ph4a.tile([128, F], bf16, tag=f"w1{k}",
                                           name=f"w1{k}")
                                 for k in range(4, KT)]
                for k in range(4, KT):
                    nc.sync.dma_start(w1_sb[k][:], w1[k * 128:(k + 1) * 128, :])
                for ft in range(FT):
                    fsl = slice(ft * 128, ft * 128 + 128)
                    ps = f1ps.tile([128, RQ], f32, tag="f1", name="f1")
                    for k in range(KT):
                        nc.tensor.matmul(ps[:], w1_sb[k][:, fsl], hT[k][:],
                                         start=(k == 0), stop=(k == KT - 1))
                    nc.scalar.activation(ff1rT[ft][:], ps[:], AF.Relu,
                                         bias=b1_sb[:, ft:ft + 1])

            stackW1.close()

            # ================= Phase 4b: FFN down + LN2 =================
            with tc.tile_pool(name="ph4b", bufs=1) as ph4b, \
                 tc.tile_pool(name="ph4w", bufs=3) as ph4w, \
                 tc.tile_pool(name="f2ps", bufs=3, space="PSUM") as f2ps:
                w2_sb = [ph4b.tile([128, D], bf16, tag=f"w2{k}", name=f"w2{k}")
                         for k in range(FT)]
                for k in range(FT):
                    nc.sync.dma_start(w2_sb[k][:], w2[k * 128:(k + 1) * 128, :])
                for qt in range(4):
                    qsl = slice(qt * 128, qt * 128 + 128)
                    ps = f2ps.tile([128, D], f32, tag="f2", name="f2")
                    for half in range(2):
                        osl = slice(half * 512, half * 512 + 512)
                        for ft in range(FT):
                            nc.tensor.matmul(ps[:, osl], ff1rT[ft][:, qsl],
                                             w2_sb[ft][:, osl],
                                             start=(ft == 0), stop=(ft == FT - 1))
                    hback = ph4w.tile([128, D], f32, tag="hback", name="hback")
                    nc.sync.dma_start(hback[:], h_dram[qsl, :])
                    res = ph4w.tile([128, D], f32, tag="res2", name="res2")
                    nc.vector.tensor_add(res[:], ps[:], hback[:])
                    if b2_rep is not None:
                        nc.vector.tensor_add(res[:], res[:], b2_rep[:])
                    o = ph4w.tile([128, D], f32, tag="out", name="out")
                    layer_norm(o[:], res[:], g2_rep, be2_rep, ph4w)
                    nc.sync.dma_start(y[qt * 128:(qt + 1) * 128, :], o[:])

    return nc



_CACHED = {}


def _get_program(flags):
    key = tuple(sorted(flags.items()))
    if key not in _CACHED:
        _CACHED[key] = _build_program(flags)
    return _CACHED[key]


def kernel(x, w_q, w_k, w_v, w_o, b_o, gamma1, beta1, gamma2, beta2,
           w1, b1, w2, b2, _trace=False):
    _install_patches()
    from concourse import bass_utils

    bf = ml_dtypes.bfloat16
    x = np.asarray(x, np.float32)
    flags = {
        "use_bo": not np.all(np.asarray(b_o) == 0),
        "use_b2": not np.all(np.asarray(b2) == 0),
        "use_g1": not np.all(np.asarray(gamma1) == 1),
        "use_be1": not np.all(np.asarray(beta1) == 0),
        "use_g2": not np.all(np.asarray(gamma2) == 1),
        "use_be2": not np.all(np.asarray(beta2) == 0),
    }
    nc = _get_program(flags)

    # host-side shared prep
    inv_freq = (1.0 / (10000.0 ** (np.arange(0, DK, 2, dtype=np.float64) / DK)))
    freqs = np.arange(L, dtype=np.float64)[:, None] * inv_freq      # [L, 32]
    cos = np.cos(freqs).T.astype(np.float32)                        # [32, L]
    sin = np.sin(freqs).T.astype(np.float32)
    cos_rep = np.tile(cos, (4, 1)).astype(bf)                       # [128, L]
    sin_sign = np.concatenate([sin, -sin, sin, -sin], 0).astype(bf)

    common = {
        "cosr": cos_rep, "sinr": sin_sign,
        "wq": w_q.astype(bf), "wk": w_k.astype(bf), "wv": w_v.astype(bf),
        "wo": w_o.astype(bf), "w1": w1.astype(bf), "w2": w2.astype(bf),
        "b1t": np.ascontiguousarray(
            np.asarray(b1, np.float32).reshape(F // 128, 128).T),
        "ident": np.eye(128, dtype=np.float32),
        "onehot": np.kron(np.eye(H, dtype=np.float32),
                          np.ones((1, 64), np.float32)).astype(bf),
        "bo": np.asarray(b_o, np.float32).reshape(1, D),
        "b2r": np.asarray(b2, np.float32).reshape(1, D),
        "g1": np.asarray(gamma1, np.float32).reshape(1, D),
        "be1": np.asarray(beta1, np.float32).reshape(1, D),
        "g2": np.asarray(gamma2, np.float32).reshape(1, D),
        "be2": np.asarray(beta2, np.float32).reshape(1, D),
    }
    xT_all = [np.ascontiguousarray(x[b].T).astype(bf) for b in range(B)]

    in_maps = []
    for c in range(NCORES):
        b, r = c // 4, c % 4
        rows = slice(r * RQ, (r + 1) * RQ)
        m = dict(common)
        m["xT"] = xT_all[b]
        m["xTq"] = np.ascontiguousarray(xT_all[b][:, rows])
        m["xr"] = np.ascontiguousarray(x[b, rows, :])
        m["qcos"] = np.ascontiguousarray(cos_rep[:, rows])
        m["qsin"] = np.ascontiguousarray(sin_sign[:, rows])
        in_maps.append(m)

    res = bass_utils.run_bass_kernel_spmd(
        nc, in_maps, core_ids=list(range(NCORES)), trace=_trace)

    out = np.empty((B, L, D), np.float32)
    for c in range(NCORES):
        b, r = c // 4, c % 4
        out[b, r * RQ:(r + 1) * RQ, :] = res.results[c]["y"]
    if _trace:
        kernel.last_exec_time_ns = res.exec_time_ns
    return out

